# revision 1
# baseline (speedup 1.0000x reference)
"""Causal self-attention Bass/Trainium2 kernel.

Problem: B=4, T=2048, D=1024, 16 heads (head_dim=64).
    qkv = x @ Wqkv + bqkv ; per-head causal softmax attention ; y @ Wo + bo

Sharding (8 cores): core = (batch b, head-group g), b = core // 2, g = core % 2.
Each core processes one batch (2048 tokens) and 8 of the 16 heads:
  - qkv_proj column-sharded by head group, out_proj row-sharded by head group
    (the 2 cores of one batch produce partial out-proj sums, summed on host).
  - x batch-sharded (and pre-transposed on host).

Everything on device lives in a transposed [feature, token] layout so no
on-device transposes are needed anywhere:
  - host feeds x^T [D, T]; Q^T/K^T [c, t] come out of the qkv matmuls directly;
  - attention scores are computed as S^T [k, q] = (K^T)^T-contraction, so the
    exp() output P^T [k, q] is already the layout the AV matmul needs;
  - softmax denominators come for free from a ones-column appended to V in the
    AV matmul's stationary operand (row 64 of the output accumulates sum_k P).
  - softmax normalization (and the V-bias fold) happen after AV: y = yU / l,
    where 1/l is broadcast across partitions with a tiny fp32r matmul.
  - out_proj emits y^T [D, T] fp32; the host transposes + sums core pairs.

exp() runs without max-subtraction: S = q.k/8 with O(1)-scale randn-derived
inputs, |S| < ~15, exp stays comfortably inside fp32/bf16 range, and softmax
is shift-invariant so the result is identical.
"""

import numpy as np
import ml_dtypes

B = 4
T = 2048
D = 1024
N_HEADS = 16
HD = 64
N_CORES = 8
G = 2                 # head groups
HL = N_HEADS // G     # heads per core (8)
CL = HL * HD          # local channel width (512)
QCH = 512             # q-chunk width (1 PSUM bank -> 2 bufs/tag)
BF16 = ml_dtypes.bfloat16

_NC_CACHE = {}


def _build_nc(t_len, add_bv):
    """Build (and bacc-compile) the single-core SPMD Bass program."""
    import concourse.bass as bass  # noqa: F401
    import concourse.tile as tile
    import concourse.mybir as mybir
    from concourse import bacc

    f32 = mybir.dt.float32
    f32r = mybir.dt.float32r
    bf16 = mybir.dt.bfloat16

    nd = D // 128            # 8 d-chunks
    ncb = CL // 128          # 4 c-blocks for Q/K
    ntb = t_len // 128       # token blocks
    qch = min(QCH, t_len)
    nqc = t_len // qch       # q chunks
    neb = D // 128           # out-proj e-blocks

    nc = bacc.Bacc("TRN2", target_bir_lowering=False, debug=False,
                   num_devices=N_CORES)

    xT = nc.dram_tensor("xT", [D, t_len], bf16, kind="ExternalInput")
    wq = nc.dram_tensor("wq", [D, CL], bf16, kind="ExternalInput")
    wk = nc.dram_tensor("wk", [D, CL], bf16, kind="ExternalInput")
    wv = nc.dram_tensor("wv", [D, CL], bf16, kind="ExternalInput")
    wo = nc.dram_tensor("wo", [CL, D], bf16, kind="ExternalInput")
    bq = nc.dram_tensor("bq", [128, ncb], f32, kind="ExternalInput")
    bk = nc.dram_tensor("bk", [128, ncb], f32, kind="ExternalInput")
    bv = nc.dram_tensor("bv", [64, HL], f32, kind="ExternalInput")
    bo = nc.dram_tensor("bo", [128, neb], f32, kind="ExternalInput")
    mask = nc.dram_tensor("mask", [128, 128], bf16, kind="ExternalInput")
    yT = nc.dram_tensor("yT", [D, t_len], f32, kind="ExternalOutput")

    Exp = mybir.ActivationFunctionType.Exp

    with tile.TileContext(nc) as tc:
        with (
            tc.tile_pool(name="const", bufs=1) as cpool,
            tc.tile_pool(name="ptp", bufs=6) as ptp,
            tc.tile_pool(name="post", bufs=3) as post,
            tc.tile_pool(name="psum", bufs=2, space="PSUM") as psp,
        ):
            # ---- persistent SBUF buffers ----
            xt_sb = [cpool.tile([128, t_len], bf16, tag=f"xt{i}", name=f"xt{i}")
                     for i in range(nd)]
            wq_sb = [cpool.tile([128, CL], bf16, tag=f"wq{i}", name=f"wq{i}")
                     for i in range(nd)]
            wk_sb = [cpool.tile([128, CL], bf16, tag=f"wk{i}", name=f"wk{i}")
                     for i in range(nd)]
            wv_sb = [cpool.tile([128, CL], bf16, tag=f"wv{i}", name=f"wv{i}")
                     for i in range(nd)]
            wo_sb = [cpool.tile([64, D], bf16, tag=f"wo{i}", name=f"wo{i}")
                     for i in range(HL)]
            qt_sb = [cpool.tile([128, t_len], bf16, tag=f"qt{i}", name=f"qt{i}")
                     for i in range(ncb)]
            kt_sb = [cpool.tile([128, t_len], bf16, tag=f"kt{i}", name=f"kt{i}")
                     for i in range(ncb)]
            # V staging: per (token-block, head) a [128, 65] block = [V_h | 1]
            vp_sb = cpool.tile([128, ntb * HL * 65], bf16, tag="vp", name="vp")
            yh_sb = [cpool.tile([64, t_len], bf16, tag=f"yh{i}", name=f"yh{i}")
                     for i in range(HL)]
            bq_sb = cpool.tile([128, ncb], f32, tag="bq", name="bq_s")
            bk_sb = cpool.tile([128, ncb], f32, tag="bk", name="bk_s")
            bv_sb = cpool.tile([64, HL], f32, tag="bv", name="bv_s")
            bo_sb = cpool.tile([128, neb], f32, tag="bo", name="bo_s")
            mask_sb = cpool.tile([128, 128], bf16, tag="mask", name="mask_s")
            ones_sb = cpool.tile([1, 64], f32, tag="ones", name="ones_s")

            # ---- input DMAs ----
            for i in range(nd):
                nc.sync.dma_start(out=xt_sb[i][:], in_=xT[i * 128:(i + 1) * 128, :])
                nc.sync.dma_start(out=wq_sb[i][:], in_=wq[i * 128:(i + 1) * 128, :])
                nc.sync.dma_start(out=wk_sb[i][:], in_=wk[i * 128:(i + 1) * 128, :])
                nc.sync.dma_start(out=wv_sb[i][:], in_=wv[i * 128:(i + 1) * 128, :])
            for i in range(HL):
                nc.sync.dma_start(out=wo_sb[i][:], in_=wo[i * 64:(i + 1) * 64, :])
            nc.sync.dma_start(out=bq_sb[:], in_=bq[:, :])
            nc.sync.dma_start(out=bk_sb[:], in_=bk[:, :])
            nc.sync.dma_start(out=bv_sb[:], in_=bv[:, :])
            nc.sync.dma_start(out=bo_sb[:], in_=bo[:, :])
            nc.sync.dma_start(out=mask_sb[:], in_=mask[:, :])
            nc.vector.memset(ones_sb[:], 1.0)
            # ones columns of the V staging buffer (col 64 of each 65-group)
            vp_ones = vp_sb[:].rearrange("p (n c) -> p n c", c=65)[:, :, 64:65]
            nc.vector.memset(vp_ones, 1.0)

            # ---- stage B: qkv projections ----
            # Q^T / K^T: [c-block 128, t]  (lhsT = w chunks, rhs = x^T)
            for dst, w_sb, b_sb in ((qt_sb, wq_sb, bq_sb), (kt_sb, wk_sb, bk_sb)):
                for cb in range(ncb):
                    for tq in range(nqc):
                        ps = psp.tile([128, qch], f32, bufs=2, tag="s",
                                      name="ps_qkv")
                        for d in range(nd):
                            for s0 in range(0, qch, 512):
                                s1 = min(qch, s0 + 512)
                                nc.tensor.matmul(
                                    ps[:, s0:s1],
                                    w_sb[d][:, cb * 128:(cb + 1) * 128],
                                    xt_sb[d][:, tq * qch + s0:tq * qch + s1],
                                    start=(d == 0), stop=(d == nd - 1),
                                )
                        nc.vector.tensor_scalar_add(
                            out=dst[cb][:, tq * qch:(tq + 1) * qch],
                            in0=ps[:],
                            scalar1=b_sb[:, cb:cb + 1],
                        )
            # V in natural [t, c] layout (lhsT = x^T chunks, rhs = wv chunk)
            for tb in range(ntb):
                ps = psp.tile([128, CL], f32, bufs=2, tag="s",
                              name="ps_v")
                for d in range(nd):
                    nc.tensor.matmul(
                        ps[:],
                        xt_sb[d][:, tb * 128:(tb + 1) * 128],
                        wv_sb[d][:],
                        start=(d == 0), stop=(d == nd - 1),
                    )
                # scatter the 8 heads' V into the staging layout
                dst = vp_sb[:].rearrange("p (n c) -> p n c", c=65)[
                    :, tb * HL:(tb + 1) * HL, 0:64]
                src = ps[:].rearrange("p (h c) -> p h c", c=64)
                nc.vector.tensor_copy(out=dst, in_=src)

            # ---- stage C: attention, head pairs interleaved ----
            # Each pair's even/odd head operands live at SBUF partition bases
            # 0/64, so their K=64 S^T matmuls auto-derive PE row groups 0/2
            # and execute concurrently in the systolic array.
            for hp in range(HL // 2):
                heads = (2 * hp, 2 * hp + 1)
                for qc in range(nqc):
                    q0 = qc * qch
                    yps = {}
                    for h in heads:
                        yps[h] = psp.tile([128, qch], f32, bufs=2,
                                          tag=f"y{h % 2}",
                                          name=f"ps_y{h % 2}")
                    njs = [j for j in range(ntb) if j * 128 < q0 + qch]
                    for j in njs:
                        qlo = max(q0, j * 128)
                        rel = qlo - q0
                        w = qch - rel
                        # both heads' S^T side by side in one 2-bank tile
                        sp = psp.tile([128, 2 * qch], f32, bufs=2, tag="s",
                                      name="ps_s")
                        for h in heads:
                            pb = (h % 2) * 64
                            qt_h = qt_sb[h // 2][pb:pb + 64, :]
                            kt_h = kt_sb[h // 2][pb:pb + 64, :]
                            nc.tensor.matmul(
                                sp[:, (h % 2) * qch + rel:(h % 2) * qch + qch],
                                kt_h[:, j * 128:(j + 1) * 128],
                                qt_h[:, qlo:q0 + qch],
                                start=True, stop=True,
                            )
                        pt = ptp.tile([128, 2 * qch], bf16, tag="pt",
                                      name="pt")
                        sp3 = sp[:].rearrange("p (n c) -> p n c", c=qch)
                        pt3 = pt[:].rearrange("p (n c) -> p n c", c=qch)
                        nc.scalar.activation(
                            out=pt3[:, :, rel:qch], in_=sp3[:, :, rel:qch],
                            func=Exp, scale=float(HD) ** -0.5,
                        )
                        if j * 128 >= q0:  # diagonal blocks: causal mask
                            m_ap = mask_sb[:]
                            m2 = bass.AP(
                                tensor=m_ap.tensor, offset=m_ap.offset,
                                ap=[list(m_ap.ap[0]), [0, 2],
                                    list(m_ap.ap[1])],
                            )
                            nc.vector.tensor_mul(
                                pt3[:, :, rel:rel + 128],
                                pt3[:, :, rel:rel + 128],
                                m2,
                            )
                        for h in heads:
                            vcol = (j * HL + h) * 65
                            nc.tensor.matmul(
                                yps[h][0:65, rel:qch],
                                vp_sb[:, vcol:vcol + 65],
                                pt[:, (h % 2) * qch + rel:(h % 2) * qch + qch],
                                start=(j == njs[0]), stop=(j == njs[-1]),
                            )
                    # normalize: y[hd, q] * (1 / l[q]) (+ folded V bias)
                    for h in heads:
                        yp = yps[h]
                        rec = post.tile([1, qch], f32, tag="rec", name="rec")
                        nc.vector.reciprocal(out=rec[:], in_=yp[64:65, :])
                        bcs = post.tile([64, qch], f32, tag="bcs", name="bcs")
                        nc.gpsimd.partition_broadcast(bcs[:], rec[:], channels=64)
                        nc.vector.tensor_mul(
                            yh_sb[h][:, q0:q0 + qch], yp[0:64, :], bcs[:],
                        )
                        if add_bv:
                            nc.vector.tensor_scalar_add(
                                out=yh_sb[h][:, q0:q0 + qch],
                                in0=yh_sb[h][:, q0:q0 + qch],
                                scalar1=bv_sb[:, h:h + 1],
                            )

            # ---- stage D: out projection (K=64 chunks, one per head) ----
            for eb in range(neb):
                for tq in range(nqc):
                    ps = psp.tile([128, qch], f32, bufs=2, tag="s",
                                  name="ps_o")
                    for h in range(HL):
                        lhsT = wo_sb[h][:, eb * 128:(eb + 1) * 128]
                        for s0 in range(0, qch, 512):
                            s1 = min(qch, s0 + 512)
                            nc.tensor.matmul(
                                ps[:, s0:s1],
                                lhsT,
                                yh_sb[h][:, tq * qch + s0:tq * qch + s1],
                                start=(h == 0), stop=(h == HL - 1),
                            )
                    ost = post.tile([128, qch], f32, tag="ost", name="ost")
                    nc.vector.tensor_scalar_add(
                        out=ost[:], in0=ps[:], scalar1=bo_sb[:, eb:eb + 1],
                    )
                    nc.sync.dma_start(
                        out=yT[eb * 128:(eb + 1) * 128, tq * qch:(tq + 1) * qch],
                        in_=ost[:],
                    )

    nc.compile()
    return nc


def get_nc(t_len=T, add_bv=False):
    key = (t_len, add_bv)
    if key not in _NC_CACHE:
        _NC_CACHE[key] = _build_nc(t_len, add_bv)
    return _NC_CACHE[key]


def make_in_maps(x, Wqkv, bqkv, Wo, bo):
    """Shard + lay out full inputs into the 8 per-core input maps."""
    x = np.asarray(x, np.float32)
    Wqkv = np.asarray(Wqkv, np.float32)
    bqkv = np.asarray(bqkv, np.float32)
    Wo = np.asarray(Wo, np.float32)
    bo = np.asarray(bo, np.float32)
    b_, t_len, d = x.shape
    mask = np.triu(np.ones((128, 128), np.float32)).astype(BF16)
    bo_t = np.ascontiguousarray(bo.reshape(D // 128, 128).T, np.float32)
    in_maps = []
    for core in range(N_CORES):
        b, g = core // G, core % G
        c0 = g * CL
        wq_s = Wqkv[:, c0:c0 + CL]
        wk_s = Wqkv[:, D + c0:D + c0 + CL]
        wv_s = Wqkv[:, 2 * D + c0:2 * D + c0 + CL]
        bq_s = bqkv[c0:c0 + CL]
        bk_s = bqkv[D + c0:D + c0 + CL]
        bv_s = bqkv[2 * D + c0:2 * D + c0 + CL]
        in_maps.append({
            "xT": np.ascontiguousarray(x[b].T).astype(BF16),
            "wq": np.ascontiguousarray(wq_s).astype(BF16),
            "wk": np.ascontiguousarray(wk_s).astype(BF16),
            "wv": np.ascontiguousarray(wv_s).astype(BF16),
            "wo": np.ascontiguousarray(Wo[c0:c0 + CL, :]).astype(BF16),
            "bq": np.ascontiguousarray(bq_s.reshape(CL // 128, 128).T, np.float32),
            "bk": np.ascontiguousarray(bk_s.reshape(CL // 128, 128).T, np.float32),
            "bv": np.ascontiguousarray(bv_s.reshape(HL, 64).T, np.float32),
            "bo": bo_t,
            "mask": np.ascontiguousarray(mask),
        })
    return in_maps


def kernel(x, Wqkv, bqkv, Wo, bo):
    from concourse.bass_utils import run_bass_kernel_spmd

    in_maps = make_in_maps(x, Wqkv, bqkv, Wo, bo)
    add_bv = bool(np.any(np.asarray(bqkv, np.float32)[2 * D:]))
    t_len = np.asarray(x).shape[1]
    nc = get_nc(t_len, add_bv)
    res = run_bass_kernel_spmd(nc, in_maps, core_ids=list(range(N_CORES)))
    outs = [r["yT"] for r in res.results]
    y = np.empty((B, t_len, D), np.float32)
    for b in range(B):
        y[b] = (outs[G * b] + outs[G * b + 1]).T
    return y



# revision 3
# speedup vs baseline: 1.1084x; 1.1084x over previous
"""Causal self-attention Bass/Trainium2 kernel.

Problem: B=4, T=2048, D=1024, 16 heads (head_dim=64).
    qkv = x @ Wqkv + bqkv ; per-head causal softmax attention ; y @ Wo + bo

Sharding (8 cores): core = (batch b, head-group g), b = core // 2, g = core % 2.
Each core processes one batch (2048 tokens) and 8 of the 16 heads:
  - qkv_proj column-sharded by head group, out_proj row-sharded by head group
    (the 2 cores of one batch produce partial out-proj sums, summed on host).
  - x batch-sharded (and pre-transposed + d-chunk-packed on host).

Device schedule (single in-order stream per engine; emission order = schedule):
  - Inputs packed into 18 DMAs, ordered so the first QKV chains start ~3us in
    and then chase the DMA stream (per-d weight chunks arrive just in time).
  - QKV 512-token chunks and attention 512-q chunks are interleaved in
    emission so the PE fills scalar-engine (exp) lag with projection matmuls.
  - Everything stays in a transposed [feature, token] layout: Q^T/K^T from the
    qkv matmuls; S^T = [k, q] scores so exp output P^T feeds AV directly; a
    ones-column in the AV stationary operand accumulates the softmax
    denominator for free (output row 64).
  - Attention j-blocks run diagonal-first so the causal mask multiply is off
    the per-chunk tail; the tail itself only copies the unnormalized [65, 512]
    AV block out of PSUM (freeing the bank), with reciprocal/broadcast/scale
    deferred off the critical path.
  - Out-proj runs head-PAIRS (K=128 matmuls, half the instructions of K=64),
    emitting bf16 partial sums; the host adds core pairs and transposes.

exp() runs without max-subtraction: S = q.k/8 with O(1)-scale randn-derived
inputs, |S| < ~15, exp stays comfortably inside fp32/bf16 range, and softmax
is shift-invariant so the result is identical.
"""

import numpy as np
import ml_dtypes

B = 4
T = 2048
D = 1024
N_HEADS = 16
HD = 64
N_CORES = 8
G = 2                 # head groups
HL = N_HEADS // G     # heads per core (8)
CL = HL * HD          # local channel width (512)
QCH = 512             # q-chunk width
BF16 = ml_dtypes.bfloat16

_NC_CACHE = {}


def _build_nc(t_len, add_bv):
    """Build (and bacc-compile) the single-core SPMD Bass program."""
    import concourse.bass as bass  # noqa: F401
    import concourse.tile as tile
    import concourse.mybir as mybir
    from concourse import bacc

    f32 = mybir.dt.float32
    bf16 = mybir.dt.bfloat16

    assert t_len % QCH == 0
    nd = D // 128            # 8 d-chunks
    ncb = CL // 128          # 4 c-blocks for Q/K
    ntb = t_len // 128       # token blocks
    nqc = t_len // QCH       # token/q chunks
    neb = D // 128           # out-proj e-blocks
    npair = HL // 2          # head pairs (4)

    nc = bacc.Bacc("TRN2", target_bir_lowering=False, debug=False,
                   num_devices=N_CORES)

    # packed inputs (column layouts documented in make_in_maps)
    xTp = nc.dram_tensor("xTp", [128, nqc * nd * QCH], bf16, kind="ExternalInput")
    wqkv = nc.dram_tensor("wqkv", [128, nd * 3 * CL], bf16, kind="ExternalInput")
    wo = nc.dram_tensor("wo", [128, npair * D], bf16, kind="ExternalInput")
    bq = nc.dram_tensor("bq", [128, ncb], f32, kind="ExternalInput")
    bk = nc.dram_tensor("bk", [128, ncb], f32, kind="ExternalInput")
    bv = nc.dram_tensor("bv", [128, npair], f32, kind="ExternalInput")
    bo = nc.dram_tensor("bo", [128, neb], f32, kind="ExternalInput")
    mask = nc.dram_tensor("mask", [128, 128], bf16, kind="ExternalInput")
    yT = nc.dram_tensor("yT", [D, t_len], bf16, kind="ExternalOutput")

    Exp = mybir.ActivationFunctionType.Exp

    with tile.TileContext(nc) as tc:
        with (
            tc.tile_pool(name="const", bufs=1) as cpool,
            tc.tile_pool(name="ptp", bufs=6) as ptp,
            tc.tile_pool(name="post", bufs=3) as post,
            tc.tile_pool(name="psum", bufs=2, space="PSUM") as psp,
        ):
            # ---- persistent SBUF buffers ----
            xts = [cpool.tile([128, nd * QCH], bf16, tag=f"xts{i}", name=f"xts{i}")
                   for i in range(nqc)]
            wq_sb = [cpool.tile([128, 3 * CL], bf16, tag=f"wqkv{i}", name=f"wqkv{i}")
                     for i in range(nd)]
            wo_sb = cpool.tile([128, npair * D], bf16, tag="wo", name="wo_s")
            qt_sb = [cpool.tile([128, t_len], bf16, tag=f"qt{i}", name=f"qt{i}")
                     for i in range(ncb)]
            kt_sb = [cpool.tile([128, t_len], bf16, tag=f"kt{i}", name=f"kt{i}")
                     for i in range(ncb)]
            # V staging: per (token-block, head) a [128, 65] block = [V_h | 1]
            vp_sb = cpool.tile([128, ntb * HL * 65], bf16, tag="vp", name="vp")
            # attention outputs per head pair: head 2p in partitions 0-63,
            # head 2p+1 in partitions 64-127 (K=128 out-proj operands)
            yh_sb = [cpool.tile([128, t_len], bf16, tag=f"yh{i}", name=f"yh{i}")
                     for i in range(npair)]
            bq_sb = cpool.tile([128, ncb], f32, tag="bq", name="bq_s")
            bk_sb = cpool.tile([128, ncb], f32, tag="bk", name="bk_s")
            bv_sb = cpool.tile([128, npair], f32, tag="bv", name="bv_s")
            bo_sb = cpool.tile([128, neb], f32, tag="bo", name="bo_s")
            mask_sb = cpool.tile([128, 128], bf16, tag="mask", name="mask_s")

            # ---- input DMAs (order = priority; compute chases the stream) ----
            nc.sync.dma_start(out=mask_sb[:], in_=mask[:, :])
            nc.sync.dma_start(out=bq_sb[:], in_=bq[:, :])
            nc.sync.dma_start(out=bk_sb[:], in_=bk[:, :])
            nc.sync.dma_start(out=bv_sb[:], in_=bv[:, :])
            nc.sync.dma_start(out=bo_sb[:], in_=bo[:, :])
            nc.sync.dma_start(out=xts[0][:], in_=xTp[:, 0:nd * QCH])
            for d in range(nd):
                nc.sync.dma_start(out=wq_sb[d][:],
                                  in_=wqkv[:, d * 3 * CL:(d + 1) * 3 * CL])
            if nqc > 1:
                nc.sync.dma_start(out=xts[1][:],
                                  in_=xTp[:, nd * QCH:2 * nd * QCH])
            nc.sync.dma_start(out=wo_sb[:], in_=wo[:, :])
            for tq in range(2, nqc):
                nc.sync.dma_start(out=xts[tq][:],
                                  in_=xTp[:, tq * nd * QCH:(tq + 1) * nd * QCH])
            # ones columns of the V staging buffer (col 64 of each 65-group)
            vp_ones = vp_sb[:].rearrange("p (n c) -> p n c", c=65)[:, :, 64:65]
            nc.vector.memset(vp_ones, 1.0)

            def qkv_chunk(tq):
                """Q^T/K^T c-blocks and V token-blocks for one 512-token chunk."""
                t0 = tq * QCH
                for dst, coff, b_sb in ((qt_sb, 0, bq_sb), (kt_sb, CL, bk_sb)):
                    for cb in range(ncb):
                        ps = psp.tile([128, QCH], f32, bufs=2, tag=f"y{cb % 2}",
                                      name="ps_qk")
                        for d in range(nd):
                            nc.tensor.matmul(
                                ps[:],
                                wq_sb[d][:, coff + cb * 128:coff + (cb + 1) * 128],
                                xts[tq][:, d * QCH:(d + 1) * QCH],
                                start=(d == 0), stop=(d == nd - 1),
                            )
                        nc.vector.tensor_scalar_add(
                            out=dst[cb][:, t0:t0 + QCH], in0=ps[:],
                            scalar1=b_sb[:, cb:cb + 1],
                        )
                for tb in range(4 * tq, 4 * tq + 4):
                    rb = (tb % 4) * 128
                    ps = psp.tile([128, CL], f32, bufs=2, tag=f"y{tb % 2}",
                                  name="ps_v")
                    for d in range(nd):
                        nc.tensor.matmul(
                            ps[:],
                            xts[tq][:, d * QCH + rb:d * QCH + rb + 128],
                            wq_sb[d][:, 2 * CL:3 * CL],
                            start=(d == 0), stop=(d == nd - 1),
                        )
                    # scatter the 8 heads' V into the staging layout
                    dst = vp_sb[:].rearrange("p (n c) -> p n c", c=65)[
                        :, tb * HL:(tb + 1) * HL, 0:64]
                    src = ps[:].rearrange("p (h c) -> p h c", c=64)
                    nc.vector.tensor_copy(out=dst, in_=src)

            def attn_chunk(hp, qc):
                """Attention for head pair hp over q in [qc*512, qc*512+512)."""
                q0 = qc * QCH
                heads = (2 * hp, 2 * hp + 1)
                yps = {h: psp.tile([128, QCH], f32, bufs=2, tag=f"y{h % 2}",
                                   name=f"ps_y{h % 2}") for h in heads}
                js = [j for j in range(ntb) if j * 128 < q0 + QCH]
                order = ([j for j in js if j * 128 >= q0]
                         + [j for j in js if j * 128 < q0])  # diagonal first
                for idx, j in enumerate(order):
                    qlo = max(q0, j * 128)
                    rel = qlo - q0
                    sp = psp.tile([128, 2 * QCH], f32, bufs=2, tag="sp",
                                  name="ps_s")
                    for h in heads:
                        pb = (h % 2) * 64
                        nc.tensor.matmul(
                            sp[:, (h % 2) * QCH + rel:(h % 2) * QCH + QCH],
                            kt_sb[hp][pb:pb + 64, j * 128:(j + 1) * 128],
                            qt_sb[hp][pb:pb + 64, qlo:q0 + QCH],
                            start=True, stop=True,
                        )
                    pt = ptp.tile([128, 2 * QCH], bf16, tag="pt", name="pt")
                    sp3 = sp[:].rearrange("p (n c) -> p n c", c=QCH)
                    pt3 = pt[:].rearrange("p (n c) -> p n c", c=QCH)
                    nc.scalar.activation(
                        out=pt3[:, :, rel:QCH], in_=sp3[:, :, rel:QCH],
                        func=Exp, scale=float(HD) ** -0.5,
                    )
                    if j * 128 >= q0:  # diagonal blocks: causal mask
                        m_ap = mask_sb[:]
                        m2 = bass.AP(
                            tensor=m_ap.tensor, offset=m_ap.offset,
                            ap=[list(m_ap.ap[0]), [0, 2], list(m_ap.ap[1])],
                        )
                        nc.vector.tensor_mul(
                            pt3[:, :, rel:rel + 128],
                            pt3[:, :, rel:rel + 128],
                            m2,
                        )
                    for h in heads:
                        vcol = (j * HL + h) * 65
                        nc.tensor.matmul(
                            yps[h][0:65, rel:QCH],
                            vp_sb[:, vcol:vcol + 65],
                            pt[:, (h % 2) * QCH + rel:(h % 2) * QCH + QCH],
                            start=(idx == 0), stop=(idx == len(order) - 1),
                        )
                # tail: drain unnormalized [65, 512] out of PSUM fast; the
                # normalize chain below is off the PE critical path.
                for h in heads:
                    hb = (h % 2) * 64
                    yu = post.tile([65, QCH], bf16, tag=f"yu{h % 2}", bufs=3,
                                   name="yu")
                    nc.vector.tensor_copy(out=yu[:], in_=yps[h][0:65, :])
                    rec = post.tile([1, QCH], f32, tag=f"rec{h % 2}", bufs=3,
                                    name="rec")
                    nc.vector.reciprocal(out=rec[:], in_=yu[64:65, :])
                    bcs = post.tile([64, QCH], f32, tag=f"bcs{h % 2}", bufs=3,
                                    name="bcs")
                    nc.gpsimd.partition_broadcast(bcs[:], rec[:], channels=64)
                    nc.vector.tensor_mul(
                        yh_sb[hp][hb:hb + 64, q0:q0 + QCH], yu[0:64, :], bcs[:],
                    )
                if add_bv:
                    nc.vector.tensor_scalar_add(
                        out=yh_sb[hp][:, q0:q0 + QCH],
                        in0=yh_sb[hp][:, q0:q0 + QCH],
                        scalar1=bv_sb[:, hp:hp + 1],
                    )

            def op_chunk(tq):
                """Out-projection for one 512-token chunk (head-pair K=128)."""
                t0 = tq * QCH
                for eb in range(neb):
                    ps = psp.tile([128, QCH], f32, bufs=2, tag=f"y{eb % 2}",
                                  name="ps_o")
                    for p in range(npair):
                        nc.tensor.matmul(
                            ps[:],
                            wo_sb[:, p * D + eb * 128:p * D + (eb + 1) * 128],
                            yh_sb[p][:, t0:t0 + QCH],
                            start=(p == 0), stop=(p == npair - 1),
                        )
                    ost = post.tile([128, QCH], bf16, tag="ost", bufs=3,
                                    name="ost")
                    nc.vector.tensor_scalar_add(
                        out=ost[:], in0=ps[:], scalar1=bo_sb[:, eb:eb + 1],
                    )
                    nc.sync.dma_start(
                        out=yT[eb * 128:(eb + 1) * 128, t0:t0 + QCH],
                        in_=ost[:],
                    )

            # ---- interleaved emission: qkv chunks fill exp lag ----
            for tq in range(nqc):
                qkv_chunk(tq)
                if tq >= 1:
                    for hp in range(npair):
                        attn_chunk(hp, tq - 1)
            for hp in range(npair):
                attn_chunk(hp, nqc - 1)
            for tq in range(nqc):
                op_chunk(tq)

    nc.compile()
    return nc


def get_nc(t_len=T, add_bv=False):
    key = (t_len, add_bv)
    if key not in _NC_CACHE:
        _NC_CACHE[key] = _build_nc(t_len, add_bv)
    return _NC_CACHE[key]


def make_in_maps(x, Wqkv, bqkv, Wo, bo):
    """Shard + lay out full inputs into the 8 per-core input maps."""
    x = np.asarray(x, np.float32)
    Wqkv = np.asarray(Wqkv, np.float32)
    bqkv = np.asarray(bqkv, np.float32)
    Wo = np.asarray(Wo, np.float32)
    bo = np.asarray(bo, np.float32)
    b_, t_len, d = x.shape
    nqc = t_len // QCH
    nd = D // 128
    npair = HL // 2
    mask = np.triu(np.ones((128, 128), np.float32)).astype(BF16)
    bo_t = np.ascontiguousarray(bo.reshape(D // 128, 128).T, np.float32)
    in_maps = []
    xp_cache = {}
    for core in range(N_CORES):
        b, g = core // G, core % G
        c0 = g * CL
        if b not in xp_cache:
            # xTp[p, (tq, d, t')] = x[b][tq*512 + t', d*128 + p]
            xp = (x[b].reshape(nqc, QCH, nd, 128)
                  .transpose(3, 0, 2, 1).reshape(128, nqc * nd * QCH))
            xp_cache[b] = np.ascontiguousarray(xp).astype(BF16)
        # wqkv[p, (d, [q|k|v], c)] : per d-chunk the local q, k, v columns
        wq_s = Wqkv[:, c0:c0 + CL]
        wk_s = Wqkv[:, D + c0:D + c0 + CL]
        wv_s = Wqkv[:, 2 * D + c0:2 * D + c0 + CL]
        wqkv_p = np.concatenate(
            [np.concatenate(
                (wq_s[d * 128:(d + 1) * 128],
                 wk_s[d * 128:(d + 1) * 128],
                 wv_s[d * 128:(d + 1) * 128]), axis=1)
             for d in range(nd)], axis=1)
        # wo[p, (pair, e)] : head-pair channel rows stacked on 128 partitions
        wo_p = np.concatenate(
            [Wo[c0 + p * 128:c0 + (p + 1) * 128, :] for p in range(npair)],
            axis=1)
        bq_s = bqkv[c0:c0 + CL]
        bk_s = bqkv[D + c0:D + c0 + CL]
        bv_s = bqkv[2 * D + c0:2 * D + c0 + CL]
        in_maps.append({
            "xTp": xp_cache[b],
            "wqkv": np.ascontiguousarray(wqkv_p).astype(BF16),
            "wo": np.ascontiguousarray(wo_p).astype(BF16),
            "bq": np.ascontiguousarray(bq_s.reshape(ncb_ := CL // 128, 128).T,
                                       np.float32),
            "bk": np.ascontiguousarray(bk_s.reshape(ncb_, 128).T, np.float32),
            "bv": np.ascontiguousarray(bv_s.reshape(npair, 128).T, np.float32),
            "bo": bo_t,
            "mask": np.ascontiguousarray(mask),
        })
    return in_maps


def kernel(x, Wqkv, bqkv, Wo, bo):
    from concourse.bass_utils import run_bass_kernel_spmd

    in_maps = make_in_maps(x, Wqkv, bqkv, Wo, bo)
    add_bv = bool(np.any(np.asarray(bqkv, np.float32)[2 * D:]))
    t_len = np.asarray(x).shape[1]
    nc = get_nc(t_len, add_bv)
    res = run_bass_kernel_spmd(nc, in_maps, core_ids=list(range(N_CORES)))
    outs = [r["yT"] for r in res.results]
    y = np.empty((B, t_len, D), np.float32)
    for b in range(B):
        y[b] = (outs[G * b].astype(np.float32)
                + outs[G * b + 1].astype(np.float32)).T
    return y


# revision 8
# speedup vs baseline: 1.2424x; 1.1209x over previous
"""Causal self-attention Bass/Trainium2 kernel.

Problem: B=4, T=2048, D=1024, 16 heads (head_dim=64).
    qkv = x @ Wqkv + bqkv ; per-head causal softmax attention ; y @ Wo + bo

Sharding (8 cores): core = (batch b, head-group g), b = core // 2, g = core % 2.
Each core processes one batch (2048 tokens) and 8 of the 16 heads:
  - qkv_proj column-sharded by head group, out_proj row-sharded by head group
    (the 2 cores of one batch produce partial out-proj sums, summed on host).
  - x batch-sharded (and pre-transposed + d-chunk-packed on host).

Device schedule (single in-order stream per engine; emission order = schedule):
  - Inputs packed into 18 DMAs, ordered so the first QKV chains start ~3us in
    and then chase the DMA stream (per-d weight chunks arrive just in time).
  - QKV 512-token chunks and attention 512-q chunks are interleaved in
    emission so the PE fills scalar-engine (exp) lag with projection matmuls.
  - Everything stays in a transposed [feature, token] layout: Q^T/K^T from the
    qkv matmuls; S^T = [k, q] scores so exp output P^T feeds AV directly; a
    ones-column in the AV stationary operand accumulates the softmax
    denominator for free (output row 64).
  - Attention j-blocks run diagonal-first so the causal mask multiply is off
    the per-chunk tail; the tail itself only copies the unnormalized [65, 512]
    AV block out of PSUM (freeing the bank), with reciprocal/broadcast/scale
    deferred off the critical path.
  - Out-proj runs head-PAIRS (K=128 matmuls, half the instructions of K=64),
    emitting bf16 partial sums; the host adds core pairs and transposes.

exp() runs without max-subtraction: S = q.k/8 with O(1)-scale randn-derived
inputs, |S| < ~15, exp stays comfortably inside fp32/bf16 range, and softmax
is shift-invariant so the result is identical.
"""

import numpy as np
import ml_dtypes

B = 4
T = 2048
D = 1024
N_HEADS = 16
HD = 64
N_CORES = 8
G = 2                 # head groups
HL = N_HEADS // G     # heads per core (8)
CL = HL * HD          # local channel width (512)
QCH = 512             # q-chunk width
BF16 = ml_dtypes.bfloat16

_NC_CACHE = {}


def _build_nc(t_len, add_bv):
    """Build (and bacc-compile) the single-core SPMD Bass program."""
    import concourse.bass as bass  # noqa: F401
    import concourse.tile as tile
    import concourse.mybir as mybir
    from concourse import bacc

    f32 = mybir.dt.float32
    bf16 = mybir.dt.bfloat16

    assert t_len % QCH == 0
    nd = D // 128            # 8 d-chunks
    ncb = CL // 128          # 4 c-blocks for Q/K
    ntb = t_len // 128       # token blocks
    nqc = t_len // QCH       # token/q chunks
    neb = D // 128           # out-proj e-blocks
    npair = HL // 2          # head pairs (4)

    nc = bacc.Bacc("TRN2", target_bir_lowering=False, debug=False,
                   num_devices=N_CORES)

    # packed inputs (column layouts documented in make_in_maps)
    xTp = nc.dram_tensor("xTp", [128, nqc * nd * QCH], bf16, kind="ExternalInput")
    wqkv = nc.dram_tensor("wqkv", [128, nd * 3 * CL], bf16, kind="ExternalInput")
    wo = nc.dram_tensor("wo", [128, npair * D], bf16, kind="ExternalInput")
    bq = nc.dram_tensor("bq", [128, ncb], f32, kind="ExternalInput")
    bk = nc.dram_tensor("bk", [128, ncb], f32, kind="ExternalInput")
    bv = nc.dram_tensor("bv", [128, npair], f32, kind="ExternalInput")
    bo = nc.dram_tensor("bo", [128, neb], f32, kind="ExternalInput")
    mask = nc.dram_tensor("mask", [128, 128], bf16, kind="ExternalInput")
    yT = nc.dram_tensor("yT", [D, t_len], bf16, kind="ExternalOutput")

    Exp = mybir.ActivationFunctionType.Exp

    with tile.TileContext(nc) as tc:
        with (
            tc.tile_pool(name="const", bufs=1) as cpool,
            tc.tile_pool(name="ptp", bufs=6) as ptp,
            tc.tile_pool(name="post", bufs=3) as post,
            tc.tile_pool(name="psum", bufs=2, space="PSUM") as psp,
        ):
            # ---- persistent SBUF buffers ----
            xts = [cpool.tile([128, nd * QCH], bf16, tag=f"xts{i}", name=f"xts{i}")
                   for i in range(nqc)]
            wq_sb = [cpool.tile([128, 3 * CL], bf16, tag=f"wqkv{i}", name=f"wqkv{i}")
                     for i in range(nd)]
            wo_sb = cpool.tile([128, npair * D], bf16, tag="wo", name="wo_s")
            qt_sb = [cpool.tile([128, t_len], bf16, tag=f"qt{i}", name=f"qt{i}")
                     for i in range(ncb)]
            kt_sb = [cpool.tile([128, t_len], bf16, tag=f"kt{i}", name=f"kt{i}")
                     for i in range(ncb)]
            # V staging: per (token-block, head) a [128, 65] block = [V_h | 1]
            vp_sb = cpool.tile([128, ntb * HL * 65], bf16, tag="vp", name="vp")
            # attention outputs per head pair: head 2p in partitions 0-63,
            # head 2p+1 in partitions 64-127 (K=128 out-proj operands)
            yh_sb = [cpool.tile([128, t_len], bf16, tag=f"yh{i}", name=f"yh{i}")
                     for i in range(npair)]
            bq_sb = cpool.tile([128, ncb], f32, tag="bq", name="bq_s")
            bk_sb = cpool.tile([128, ncb], f32, tag="bk", name="bk_s")
            bv_sb = cpool.tile([128, npair], f32, tag="bv", name="bv_s")
            bo_sb = cpool.tile([128, neb], f32, tag="bo", name="bo_s")
            mask_sb = cpool.tile([128, 128], bf16, tag="mask", name="mask_s")

            # ---- input DMAs (order = priority; compute chases the stream) ----
            nc.sync.dma_start(out=xts[0][:], in_=xTp[:, 0:nd * QCH])
            nc.sync.dma_start(out=wq_sb[0][:], in_=wqkv[:, 0:3 * CL])
            nc.sync.dma_start(out=bq_sb[:], in_=bq[:, :])
            nc.sync.dma_start(out=bk_sb[:], in_=bk[:, :])
            for d in range(1, nd):
                nc.sync.dma_start(out=wq_sb[d][:],
                                  in_=wqkv[:, d * 3 * CL:(d + 1) * 3 * CL])
            nc.sync.dma_start(out=mask_sb[:], in_=mask[:, :])
            for tq in range(1, nqc):
                nc.sync.dma_start(out=xts[tq][:],
                                  in_=xTp[:, tq * nd * QCH:(tq + 1) * nd * QCH])
            nc.sync.dma_start(out=bv_sb[:], in_=bv[:, :])
            nc.sync.dma_start(out=bo_sb[:], in_=bo[:, :])
            nc.sync.dma_start(out=wo_sb[:], in_=wo[:, :])
            # ones columns of the V staging buffer (col 64 of each 65-group)
            vp_ones = vp_sb[:].rearrange("p (n c) -> p n c", c=65)[:, :, 64:65]
            nc.vector.memset(vp_ones, 1.0)

            def qk_piece(tq, cb, coff, dst, b_sb):
                t0 = tq * QCH
                ps = psp.tile([128, QCH], f32, bufs=2, tag=f"y{cb % 2}",
                              name="ps_qk")
                for d in range(nd):
                    nc.tensor.matmul(
                        ps[:],
                        wq_sb[d][:, coff + cb * 128:coff + (cb + 1) * 128],
                        xts[tq][:, d * QCH:(d + 1) * QCH],
                        start=(d == 0), stop=(d == nd - 1),
                    )
                nc.vector.tensor_scalar_add(
                    out=dst[cb][:, t0:t0 + QCH], in0=ps[:],
                    scalar1=b_sb[:, cb:cb + 1],
                )

            def v_piece(tb):
                rb = (tb % 4) * 128
                ps = psp.tile([128, CL], f32, bufs=2, tag=f"y{tb % 2}",
                              name="ps_v")
                for d in range(nd):
                    nc.tensor.matmul(
                        ps[:],
                        xts[tb // 4][:, d * QCH + rb:d * QCH + rb + 128],
                        wq_sb[d][:, 2 * CL:3 * CL],
                        start=(d == 0), stop=(d == nd - 1),
                    )
                # scatter the 8 heads' V into the staging layout
                dst = vp_sb[:].rearrange("p (n c) -> p n c", c=65)[
                    :, tb * HL:(tb + 1) * HL, 0:64]
                src = ps[:].rearrange("p (h c) -> p h c", c=64)
                nc.vector.tensor_copy(out=dst, in_=src)

            def qkv_pieces(tq):
                """PE-chain closures for one 512-token qkv chunk (12 pieces)."""
                out = []
                for dst, coff, b_sb in ((qt_sb, 0, bq_sb), (kt_sb, CL, bk_sb)):
                    for cb in range(ncb):
                        out.append((tq, lambda tq=tq, cb=cb, coff=coff,
                                    dst=dst, b_sb=b_sb:
                                    qk_piece(tq, cb, coff, dst, b_sb)))
                for tb in range(4 * tq, 4 * tq + 4):
                    out.append((tq, lambda tb=tb: v_piece(tb)))
                return out

            def attn_chunk(hp, qc, filler):
                """Attention for head pair hp over q in [qc*512, qc*512+512).

                ``filler`` is a deque of (tq, closure) PE-work pieces; one is
                emitted after the first S block and then every 6 j-blocks so
                the PE absorbs the scalar engine's per-exp overhead lag.
                """
                q0 = qc * QCH
                heads = (2 * hp, 2 * hp + 1)
                yps = {h: psp.tile([128, QCH], f32, bufs=2, tag=f"y{h % 2}",
                                   name=f"ps_y{h % 2}") for h in heads}
                js = [j for j in range(ntb) if j * 128 < q0 + QCH]
                order = ([j for j in js if j * 128 >= q0]
                         + [j for j in js if j * 128 < q0])  # diagonal first
                for idx, j in enumerate(order):
                    qlo = max(q0, j * 128)
                    rel = qlo - q0
                    sp = psp.tile([128, 2 * QCH], f32, bufs=2, tag="sp",
                                  name="ps_s")
                    for h in heads:
                        pb = (h % 2) * 64
                        nc.tensor.matmul(
                            sp[:, (h % 2) * QCH + rel:(h % 2) * QCH + QCH],
                            kt_sb[hp][pb:pb + 64, j * 128:(j + 1) * 128],
                            qt_sb[hp][pb:pb + 64, qlo:q0 + QCH],
                            start=True, stop=True,
                        )
                    pt = ptp.tile([128, 2 * QCH], bf16, tag="pt", name="pt")
                    sp3 = sp[:].rearrange("p (n c) -> p n c", c=QCH)
                    pt3 = pt[:].rearrange("p (n c) -> p n c", c=QCH)
                    nc.scalar.activation(
                        out=pt3[:, :, rel:QCH], in_=sp3[:, :, rel:QCH],
                        func=Exp, scale=float(HD) ** -0.5,
                    )
                    # PE filler between S and AV: absorbs exp latency/lag
                    if idx % 6 == 0 and filler:
                        filler.popleft()[1]()
                    if j * 128 >= q0:  # diagonal blocks: causal mask
                        m_ap = mask_sb[:]
                        m2 = bass.AP(
                            tensor=m_ap.tensor, offset=m_ap.offset,
                            ap=[list(m_ap.ap[0]), [0, 2], list(m_ap.ap[1])],
                        )
                        nc.vector.tensor_mul(
                            pt3[:, :, rel:rel + 128],
                            pt3[:, :, rel:rel + 128],
                            m2,
                        )
                    for h in heads:
                        vcol = (j * HL + h) * 65
                        nc.tensor.matmul(
                            yps[h][0:65, rel:QCH],
                            vp_sb[:, vcol:vcol + 65],
                            pt[:, (h % 2) * QCH + rel:(h % 2) * QCH + QCH],
                            start=(idx == 0), stop=(idx == len(order) - 1),
                        )
                # tail: drain unnormalized [65, 512] out of PSUM fast; the
                # normalize chain below is off the PE critical path.
                for h in heads:
                    hb = (h % 2) * 64
                    yu = post.tile([65, QCH], bf16, tag=f"yu{h % 2}", bufs=3,
                                   name="yu")
                    nc.vector.tensor_copy(out=yu[:], in_=yps[h][0:65, :])
                    rec = post.tile([1, QCH], f32, tag=f"rec{h % 2}", bufs=3,
                                    name="rec")
                    nc.vector.reciprocal(out=rec[:], in_=yu[64:65, :])
                    bcs = post.tile([64, QCH], f32, tag=f"bcs{h % 2}", bufs=3,
                                    name="bcs")
                    nc.gpsimd.partition_broadcast(bcs[:], rec[:], channels=64)
                    nc.vector.tensor_mul(
                        yh_sb[hp][hb:hb + 64, q0:q0 + QCH], yu[0:64, :], bcs[:],
                    )
                if add_bv:
                    nc.vector.tensor_scalar_add(
                        out=yh_sb[hp][:, q0:q0 + QCH],
                        in0=yh_sb[hp][:, q0:q0 + QCH],
                        scalar1=bv_sb[:, hp:hp + 1],
                    )

            def op_piece(tq, eb, use_act):
                """Out-projection e-block for one token chunk (K=128 pairs)."""
                t0 = tq * QCH
                ps = psp.tile([128, QCH], f32, bufs=2, tag=f"y{eb % 2}",
                              name="ps_o")
                for p in range(npair):
                    nc.tensor.matmul(
                        ps[:],
                        wo_sb[:, p * D + eb * 128:p * D + (eb + 1) * 128],
                        yh_sb[p][:, t0:t0 + QCH],
                        start=(p == 0), stop=(p == npair - 1),
                    )
                ost = post.tile([128, QCH], bf16, tag="ost", bufs=4,
                                name="ost")
                if use_act:
                    nc.scalar.activation(
                        out=ost[:], in_=ps[:],
                        func=mybir.ActivationFunctionType.Identity,
                        bias=bo_sb[:, eb:eb + 1],
                    )
                else:
                    nc.vector.tensor_scalar_add(
                        out=ost[:], in0=ps[:], scalar1=bo_sb[:, eb:eb + 1],
                    )
                nc.sync.dma_start(
                    out=yT[eb * 128:(eb + 1) * 128, t0:t0 + QCH],
                    in_=ost[:],
                )

            def op_pieces(tq):
                return [(nqc, lambda tq=tq, eb=eb: op_piece(tq, eb, False))
                        for eb in range(neb)]

            # ---- interleaved emission ----
            # qkv chunk 0 runs first (chasing the input DMA stream); later
            # qkv chunks and finished token-chunks' out-projections feed the
            # filler queue consumed inside attention chunks.
            from collections import deque
            filler = deque()
            for tq, piece in qkv_pieces(0):
                piece()
            for tq in range(1, nqc):
                filler.extend(qkv_pieces(tq))
            for qc in range(nqc):
                # attention for qc needs qkv(tq <= qc) flushed from the queue
                while filler and filler[0][0] <= qc:
                    filler.popleft()[1]()
                for hp in range(npair):
                    attn_chunk(hp, qc, filler)
                    if filler:
                        filler.popleft()[1]()
                filler.extend(op_pieces(qc))
            while filler:
                filler.popleft()[1]()

    nc.compile()
    return nc


def get_nc(t_len=T, add_bv=False):
    key = (t_len, add_bv)
    if key not in _NC_CACHE:
        _NC_CACHE[key] = _build_nc(t_len, add_bv)
    return _NC_CACHE[key]


def make_in_maps(x, Wqkv, bqkv, Wo, bo):
    """Shard + lay out full inputs into the 8 per-core input maps."""
    x = np.asarray(x, np.float32)
    Wqkv = np.asarray(Wqkv, np.float32)
    bqkv = np.asarray(bqkv, np.float32)
    Wo = np.asarray(Wo, np.float32)
    bo = np.asarray(bo, np.float32)
    b_, t_len, d = x.shape
    nqc = t_len // QCH
    nd = D // 128
    npair = HL // 2
    mask = np.triu(np.ones((128, 128), np.float32)).astype(BF16)
    bo_t = np.ascontiguousarray(bo.reshape(D // 128, 128).T, np.float32)
    in_maps = []
    xp_cache = {}
    for core in range(N_CORES):
        b, g = core // G, core % G
        c0 = g * CL
        if b not in xp_cache:
            # xTp[p, (tq, d, t')] = x[b][tq*512 + t', d*128 + p]
            xp = (x[b].reshape(nqc, QCH, nd, 128)
                  .transpose(3, 0, 2, 1).reshape(128, nqc * nd * QCH))
            xp_cache[b] = np.ascontiguousarray(xp).astype(BF16)
        # wqkv[p, (d, [q|k|v], c)] : per d-chunk the local q, k, v columns
        wq_s = Wqkv[:, c0:c0 + CL]
        wk_s = Wqkv[:, D + c0:D + c0 + CL]
        wv_s = Wqkv[:, 2 * D + c0:2 * D + c0 + CL]
        wqkv_p = np.concatenate(
            [np.concatenate(
                (wq_s[d * 128:(d + 1) * 128],
                 wk_s[d * 128:(d + 1) * 128],
                 wv_s[d * 128:(d + 1) * 128]), axis=1)
             for d in range(nd)], axis=1)
        # wo[p, (pair, e)] : head-pair channel rows stacked on 128 partitions
        wo_p = np.concatenate(
            [Wo[c0 + p * 128:c0 + (p + 1) * 128, :] for p in range(npair)],
            axis=1)
        bq_s = bqkv[c0:c0 + CL]
        bk_s = bqkv[D + c0:D + c0 + CL]
        bv_s = bqkv[2 * D + c0:2 * D + c0 + CL]
        in_maps.append({
            "xTp": xp_cache[b],
            "wqkv": np.ascontiguousarray(wqkv_p).astype(BF16),
            "wo": np.ascontiguousarray(wo_p).astype(BF16),
            "bq": np.ascontiguousarray(bq_s.reshape(ncb_ := CL // 128, 128).T,
                                       np.float32),
            "bk": np.ascontiguousarray(bk_s.reshape(ncb_, 128).T, np.float32),
            "bv": np.ascontiguousarray(bv_s.reshape(npair, 128).T, np.float32),
            "bo": bo_t,
            "mask": np.ascontiguousarray(mask),
        })
    return in_maps


def kernel(x, Wqkv, bqkv, Wo, bo):
    from concourse.bass_utils import run_bass_kernel_spmd

    in_maps = make_in_maps(x, Wqkv, bqkv, Wo, bo)
    add_bv = bool(np.any(np.asarray(bqkv, np.float32)[2 * D:]))
    t_len = np.asarray(x).shape[1]
    nc = get_nc(t_len, add_bv)
    res = run_bass_kernel_spmd(nc, in_maps, core_ids=list(range(N_CORES)))
    outs = [r["yT"] for r in res.results]
    y = np.empty((B, t_len, D), np.float32)
    for b in range(B):
        y[b] = (outs[G * b].astype(np.float32)
                + outs[G * b + 1].astype(np.float32)).T
    return y
